# revision 57
# baseline (speedup 1.0000x reference)
"""InfoNCE lower-bound kernel for 8 Trainium2 NeuronCores (v3).

Math (reference):
  hx = x @ W1x.T ; hy = y @ W1y.T            [N, H]
  z_ij = relu(hx[j] + hy[i] + b1) . w2       (logit WITHOUT b2)
  T1[i,j] = softplus(z_ij + b2)
  T0[i]   = T1[i,i]
  lse[i]  = log(sum_j exp(T1[i,j])) = log(N + sum_j exp(z_ij + b2))
  out     = mean(T0) - (mean(lse) - log N)

Sharding: data-parallel over i (rows of the pair grid); each core gets 64
rows of y, x and params replicated.

v3 design (on top of the v2 fp32->bf16 baseline, z engine unchanged from
v2: 4 psum strips at tile_position (0,32c), M=16 w2 q-blocks — column-
tiled matmuls overlap in the PE array for ~132-150ns effective per
F=512 z matvec):
  * All inputs pre-formatted on the host into exact SBUF layouts so every
    input DMA is a flat [128, F] contiguous copy; issues are spread across
    both HWDGE engines (sync + scalar). Each DMA issue costs ~600ns of
    sequencer time regardless of size, so transfers are consolidated.
  * PE p-state warmup: dummy [128,64] matmuls burn the slow p-state window
    while input DMAs are in flight (more than ~32 just delays dispatch —
    the PE sequencer feeds ~140ns/instr).
  * All preamble PSUM->SBUF copies go through Act (idle then), keeping the
    DVE queue pure relu work.
  * exp uses the Act accumulator (accum_out) to fuse the free-axis
    sum_j exp(z+b2) into the activation op — no DVE reduces at all. The
    last block drains strip-major so each strip's exp pipelines behind
    the remaining matmuls, and its relu pattern is Act-light (2A) since
    Act also runs the 4 exp+accum tail ops.
  * The diag path (T0 logits) runs entirely on PE + Act: b1 via a rank-1
    K=1 matmul, hy added via an fp32 identity matmul, relu from PSUM on
    Act; spread one h-tile per block over blocks 2-4.
  * Tried and reverted (all measured slower on HW): fp8 DoubleRow z
    matmuls (forbids the column tiling, DR itself 445ns vs 2x216;
    LdWeights dual-fp8 needs 16B-aligned APs with slot stride %16==0),
    single/dual-bank M=64/M=32 accumulation at one PE position (same-
    region back-to-back accumulate costs ~70ns/matmul), gpsimd relu
    offload (7.5-15us/op + slows concurrent DVE ops ~20%), fp8 relu
    outputs on DVE (drops 4x mode to 2x: 396ns vs 264).
  * Device ships per-row partial results (sum_j exp(z+b2) and diag
    logits); the final ln/softplus/means run on the host.
"""

import math

import numpy as np

N = 512
XD = 768
YD = 768
H = 300
NCORES = 8
ISH = N // NCORES   # 64 rows per core
KD = XD // 128      # 6 contraction tiles of 128
HT = 3              # h tiles: 128, 128, 44
HSZ = [128, 128, H - 256]

# Engine assignment for the 20 relu ops per block of 8 rows, in emission
# order [8x t0, 8x t1, 4x t2-pair]. 'D' = DVE (vector), 'A' = Activation
# (scalar). fp8 DoubleRow was tried and REVERTED: DoubleRow forbids the
# tile_position column tiling below, and that tiling is worth more — PE
# matmuls on different column quadrants genuinely overlap (measured
# ~171ns effective per z matmul vs 216 without, and DR itself measured
# 445ns vs its 2x216 replacement).
PATTERN20_6A = (['D', 'A', 'D', 'D', 'D', 'D', 'A', 'D'] * 2 + ['A', 'A', 'D', 'D'])
PATTERN20_5A = (['D', 'A', 'D', 'D', 'D', 'D', 'A', 'D'] * 2 + ['A', 'D', 'D', 'D'])
# Last block: Act must also run the 4 exp+accum tail ops, so it gets only
# 2 relus (early strips) and the DVE absorbs the rest.
PATTERN20_2A = (['D', 'A', 'D', 'D', 'D', 'A', 'D', 'D'] + ['D'] * 8 + ['D', 'D', 'D', 'D'])

# w2a_sb (bf16) column offsets: diag cols 0:4, then 40 16-col q-blocks:
# sec0 = t0 (q=0..15), sec1 = t1, sec2 = t2 pairs (m=0..7: w2 tail in col
# 2m rows 0:44 and col 2m+1 rows 64:108).
W2A_W = 4 + 40 * 16

_CACHE = {}
TRACE = False
LAST_RESULTS = None


def _build_module():
    import concourse.bacc as bacc
    import concourse.mybir as mybir
    from concourse.tile import TileContext

    f32 = mybir.dt.float32
    bf16 = mybir.dt.bfloat16
    AF = mybir.ActivationFunctionType
    ALU = mybir.AluOpType

    nc = bacc.Bacc("TRN2", target_bir_lowering=False, debug=False)

    # Per-core inputs (SPMD: same shapes, different data for yt/xtd slices).
    xt_p = nc.dram_tensor("xt_p", [128, KD * N], bf16, kind="ExternalInput")
    w1x_p = nc.dram_tensor("w1x_p", [128, KD * H], bf16, kind="ExternalInput")
    w1y_p = nc.dram_tensor("w1y_p", [128, KD * H], bf16, kind="ExternalInput")
    aux_p = nc.dram_tensor("aux_p", [128, 2 * KD * ISH + W2A_W], bf16, kind="ExternalInput")
    bcons = nc.dram_tensor("bcons", [128, HT + 1 + 128], f32, kind="ExternalInput")
    b1row = nc.dram_tensor("b1row", [1, 3 * 128], bf16, kind="ExternalInput")
    outS = nc.dram_tensor("outS", [128, 4], f32, kind="ExternalOutput")  # accum image
    outD = nc.dram_tensor("outD", [1, ISH], f32, kind="ExternalOutput")  # diag logits

    AUX_YT, AUX_XTD, AUX_W2A = 0, KD * ISH, 2 * KD * ISH

    with TileContext(nc) as tc:
        cpool = tc.alloc_tile_pool(name="consts", bufs=1)
        rpool = tc.alloc_tile_pool(name="work", bufs=32)
        pp_pre = tc.alloc_tile_pool(name="pp_pre", bufs=1, space="PSUM")
        pp_z = tc.alloc_tile_pool(name="pp_z", bufs=1, space="PSUM")
        pp_d = tc.alloc_tile_pool(name="pp_d", bufs=1, space="PSUM")

        # ---- constant tiles ----
        xt_sb = cpool.tile([128, KD * N], bf16, tag="xt")
        w1x_sb = cpool.tile([128, KD * H], bf16, tag="w1x")
        w1y_sb = cpool.tile([128, KD * H], bf16, tag="w1y")
        aux_sb = cpool.tile([128, 2 * KD * ISH + W2A_W], bf16, tag="aux")
        bc_sb = cpool.tile([128, HT + 1 + 128], f32, tag="bc")
        b1r_sb = cpool.tile([1, 3 * 128], bf16, tag="b1r")

        hxb0 = cpool.tile([128, N], bf16, tag="hxb0")    # relu-arg x part (+b1), t0
        hxb1 = cpool.tile([128, N], bf16, tag="hxb1")    # t1
        hxb2p = cpool.tile([128, N], bf16, tag="hxb2p")  # t2 packed (rows 0:44, 64:108)
        hyf_sb = cpool.tile([128, HT * ISH], f32, tag="hyf")   # hy per h-tile (f32)
        hy2p = cpool.tile([128, ISH // 2], f32, tag="hy2p")    # packed t2 pairs
        ones64 = cpool.tile([1, ISH], bf16, tag="ones64")
        escr = cpool.tile([128, 4 * N], bf16, tag="escr")      # exp out (unread)
        sexp4 = cpool.tile([128, 4], f32, tag="sexp4")
        dlog = cpool.tile([1, ISH], f32, tag="dlog")
        warm = cpool.tile([128, 64], bf16, tag="warm")

        def w2a(lo, hi):
            return aux_sb[:, AUX_W2A + lo:AUX_W2A + hi]

        # ---- input DMAs ----
        def half(eng, dst, src, h):
            w = dst.shape[1] // 2
            eng.dma_start(dst[:, h * w:(h + 1) * w], src[:, h * w:(h + 1) * w])
        half(nc.sync, xt_sb, xt_p, 0)
        half(nc.scalar, w1x_sb, w1x_p, 0)
        half(nc.sync, xt_sb, xt_p, 1)
        half(nc.scalar, w1x_sb, w1x_p, 1)
        nc.sync.dma_start(aux_sb[:], aux_p[:])
        nc.scalar.dma_start(w1y_sb[:], w1y_p[:])
        nc.sync.dma_start(bc_sb[:], bcons[:])
        nc.scalar.dma_start(b1r_sb[:], b1row[:])

        # ---- PE p-state warmup: burn the slow-clock window on dummy
        # matmuls (no input deps) while the DMAs land ----
        nc.gpsimd.memset(warm[:], 0.0)
        # shares the diag dps buffer (same pool/tag/shape) — psum is full
        wps = pp_d.tile([128, 64], f32, tag="dps", name="warm_ps")
        for _ in range(32):
            nc.tensor.matmul(wps[0:64, :], lhsT=warm[:, 0:64], rhs=warm[:],
                             start=True, stop=True, skip_group_check=True)

        nc.gpsimd.memset(ones64[:], 1.0)
        # zero the packed-t2 operand tiles before their writers fill the
        # live rows, so the pair matmul's zero-weight rows multiply finite
        # values (NaN * 0 = NaN).
        nc.gpsimd.memset(hxb2p[:], 0.0)
        nc.gpsimd.memset(hy2p[:], 0.0)

        # ---- preamble: hxb (x part, +b1) and hy per h-tile, with the hx
        # and hy k-matmuls interleaved (alternating psum regions dodges the
        # same-region back-to-back accumulate penalty, and keeps hy-t0 from
        # being scheduled after all the hx tiles — hyf t0 gates the first
        # relus). All PSUM->SBUF copies go through Act (idle here) so the
        # DVE queue is pure relu work. ----
        for t in range(HT):
            hs = HSZ[t]
            ps = pp_pre.tile([128, N], f32, tag="pre512", bufs=2)
            for k in range(KD):
                nc.tensor.matmul(
                    ps[0:hs, :],
                    lhsT=w1x_sb[:, k * H + 128 * t: k * H + 128 * t + hs],
                    rhs=xt_sb[:, k * N:(k + 1) * N],
                    start=(k == 0), stop=(k == KD - 1),
                )
            dst = [hxb0, hxb1, hxb2p][t]
            nc.scalar.activation(
                dst[0:hs, :], ps[0:hs, :], AF.Identity, bias=bc_sb[0:hs, t:t + 1]
            )
            psy = pp_pre.tile([128, ISH], f32, tag="pre64", bufs=1)
            for k in range(KD):
                nc.tensor.matmul(
                    psy[0:hs, :],
                    lhsT=w1y_sb[:, k * H + 128 * t: k * H + 128 * t + hs],
                    rhs=aux_sb[:, AUX_YT + k * ISH:AUX_YT + (k + 1) * ISH],
                    start=(k == 0), stop=(k == KD - 1),
                )
            nc.scalar.activation(
                hyf_sb[0:hs, t * ISH:(t + 1) * ISH], psy[0:hs, :], AF.Identity
            )
            if t == 2:
                # packed pair layout: col p <- (even col 2p at rows 0:44,
                # odd col 2p+1 at rows 64:108)
                evens = psy[0:hs, :].rearrange("p (a two) -> p two a", two=2)
                nc.scalar.activation(hy2p[0:hs, :], evens[:, 0, :], AF.Identity)
                nc.scalar.activation(hy2p[64:64 + hs, :], evens[:, 1, :], AF.Identity)
        nc.vector.tensor_copy(hxb2p[64:64 + HSZ[2], :], hxb2p[0:HSZ[2], :])

        # ---- main loop: 8 blocks of 8 rows, v2's 4-strip z engine ----
        # row i -> strip c=(i//2)%4, psum row 32c + q, q = 2*(i//8) + i%2.
        # Consecutive z matmuls target different PE column quadrants
        # (tile_position (0,32c)) and different psum banks, which lets the
        # PE overlap their streams (~171ns effective vs 216 serial).
        def relu_op(eng, out_ap, in_ap, col_f32):
            if eng == 'A':
                nc.scalar.activation(out_ap, in_ap, AF.Relu, bias=col_f32)
            else:
                nc.vector.tensor_scalar(out_ap, in_ap, col_f32, 0.0, ALU.add, ALU.max)

        def w2q_blk(sec, idx):
            off = AUX_W2A + 4 + (sec * 16 + idx) * 16
            return aux_sb[:, off:off + 16]

        zbk = [
            pp_z.tile([128, N], f32, tag=f"zp{c}", name=f"zp{c}") for c in range(4)
        ]

        NB = ISH // 8
        diag_dps = {}
        rps_store = {}   # (bsrc, c) -> t2 pair tile

        def emit_t2_relu(bsrc, strips):
            patt = PATTERN20_6A if bsrc < 6 else PATTERN20_5A
            for c in strips:
                rp = rpool.tile([128, N], bf16, tag="rp", bufs=10,
                                name=f"rp_{bsrc}_{c}")
                relu_op(patt[16 + c], rp[:], hxb2p[:],
                        hy2p[:, 4 * bsrc + c:4 * bsrc + c + 1])
                rps_store[(bsrc, c)] = rp

        def zpair(bsrc, c):
            nc.tensor.matmul(
                zbk[c][32 * c:32 * c + 16, :],
                lhsT=w2q_blk(2, bsrc), rhs=rps_store[(bsrc, c)][:],
                start=False, stop=(bsrc == NB - 1),
                tile_position=(0, 32 * c),
                skip_group_check=True,
            )

        # NOTE: offloading a few t2 tiles to gpsimd was tried and REVERTED:
        # Pool's tensor_scalar ucode took 7.5-15us per [128,512] op AND its
        # SBUF traffic slowed every concurrent DVE op by ~70%.
        for b in range(NB):
            last = b == NB - 1
            PATTERN20 = PATTERN20_6A if b < 6 else (PATTERN20_5A if b == 6 else PATTERN20_2A)
            iord = [8 * b + 2 * c + j for j in range(2) for c in range(4)]
            if not last:
                tslots = [(i, 0) for i in iord] + [(i, 1) for i in iord]
            else:
                tslots = []
                for c in range(4):
                    i0 = 8 * b + 2 * c
                    tslots += [(i0, 0), (i0 + 1, 0), (i0, 1), (i0 + 1, 1)]
            t2_own = [0, 1, 2, 3]

            rt = {}
            for slot, (idx, t) in enumerate(tslots):
                r = rpool.tile([128, N], bf16, tag="r", bufs=32)
                relu_op(
                    PATTERN20[slot], r[:], [hxb0, hxb1][t][:],
                    hyf_sb[:, t * ISH + idx: t * ISH + idx + 1],
                )
                rt[(idx, t)] = r
            emit_t2_relu(b, t2_own)

            def zmm(i, t):
                c = (i // 2) % 4
                q = 2 * (i // 8) + (i % 2)
                nc.tensor.matmul(
                    zbk[c][32 * c:32 * c + 16, :],
                    lhsT=w2q_blk(t, q), rhs=rt[(i, t)][:],
                    start=(b == 0 and t == 0 and i % 2 == 0), stop=False,
                    tile_position=(0, 32 * c),
                    skip_group_check=True,
                )

            if not last:
                for t in range(2):
                    for i in iord:
                        zmm(i, t)
                for c in range(4):
                    zpair(b, c)
            else:
                # strip-major drain: each strip's stop fires early so its
                # exp(+fused accumulate sum_j) pipelines behind the rest.
                for c in range(4):
                    i0 = 8 * b + 2 * c
                    zmm(i0, 0)
                    zmm(i0 + 1, 0)
                    zmm(i0, 1)
                    zmm(i0 + 1, 1)
                    zpair(b, c)
                    nc.scalar.activation(
                        escr[:, c * N:(c + 1) * N], zbk[c][:], AF.Exp,
                        bias=bc_sb[:, HT:HT + 1],
                        accum_out=sexp4[:, c:c + 1],
                    )

            if b in (2, 3, 4):
                # ---- diag: dlog[i] = w2 . relu(hxd_i + hy_i + b1), all on
                # PE + Act (b1 via a rank-1 K=1 matmul, hy via an fp32
                # identity matmul, relu straight from PSUM on Act), one
                # h-tile per block to spread the Act/PE load; starts at
                # b==2 so the aux/b1row DMAs land after grid start ----
                dt_ = b - 2
                if dt_ == 0:
                    dps = pp_d.tile([128, ISH], f32, tag="dps")
                    diag_dps[0] = dps
                dps = diag_dps[0]
                hs = HSZ[dt_]
                psd = pp_pre.tile([128, ISH], f32, tag="pre64", bufs=1)
                for k in range(KD):
                    nc.tensor.matmul(
                        psd[0:hs, :],
                        lhsT=w1x_sb[:, k * H + 128 * dt_: k * H + 128 * dt_ + hs],
                        rhs=aux_sb[:, AUX_XTD + k * ISH:AUX_XTD + (k + 1) * ISH],
                        start=(k == 0), stop=False,
                    )
                nc.tensor.matmul(
                    psd[0:hs, :],
                    lhsT=b1r_sb[0:1, 128 * dt_:128 * dt_ + hs],
                    rhs=ones64[0:1, :],
                    start=False, stop=False,
                )
                nc.tensor.matmul(
                    psd[0:hs, :],
                    lhsT=bc_sb[0:hs, 4:4 + hs],
                    rhs=hyf_sb[0:hs, dt_ * ISH:(dt_ + 1) * ISH],
                    start=False, stop=True,
                )
                dr = rpool.tile([128, ISH], bf16, tag="dr", bufs=2)
                nc.scalar.activation(dr[0:hs, :], psd[0:hs, :], AF.Relu)
                dcol = AUX_W2A + (dt_ if dt_ < 2 else 2)
                nc.tensor.matmul(
                    dps[0:1, :],
                    lhsT=aux_sb[0:hs, dcol:dcol + 1],
                    rhs=dr[0:hs, :],
                    start=(dt_ == 0), stop=(dt_ == HT - 1),
                )
                if dt_ == HT - 1:
                    nc.scalar.activation(dlog[0:1, :], dps[0:1, :], AF.Identity)
                    nc.sync.dma_start(outD[0:1, :], dlog[0:1, :])

        # one output DMA for all 4 strips' accumulators; host decodes
        # S for local i from row 32c+q of col c.
        nc.sync.dma_start(outS[:, :], sexp4[:, :])

        for p in (pp_d, pp_z, pp_pre, rpool, cpool):
            p.release()

    nc.finalize()
    return nc


def _get_module():
    if "nc" not in _CACHE:
        _CACHE["nc"] = _build_module()
    return _CACHE["nc"]


def kernel(**inputs) -> np.ndarray:
    import ml_dtypes
    from concourse.bass_utils import run_bass_kernel_spmd

    bf = ml_dtypes.bfloat16
    x = np.ascontiguousarray(np.asarray(inputs["x_samples"], dtype=np.float32))
    y = np.ascontiguousarray(np.asarray(inputs["y_samples"], dtype=np.float32))
    W1 = np.asarray(inputs["W1"], dtype=np.float32)
    b1 = np.asarray(inputs["b1"], dtype=np.float32).reshape(H)
    W2 = np.asarray(inputs["W2"], dtype=np.float32)
    b2 = float(np.asarray(inputs["b2"], dtype=np.float32).reshape(1)[0])

    def sbuf_fmt(aT):
        # [KD*128, F] transposed matrix -> SBUF tile layout [128, KD*F]
        # (row p, col k*F+f = aT[128k+p, f]), contiguous.
        kd = aT.shape[0] // 128
        return np.ascontiguousarray(
            aT.reshape(kd, 128, -1).transpose(1, 0, 2).reshape(128, -1).astype(bf)
        )

    xt_p = sbuf_fmt(x.T)             # [128, 6*512]
    w1x_p = sbuf_fmt(W1[:, :XD].T)   # [128, 6*300]
    w1y_p = sbuf_fmt(W1[:, XD:].T)   # [128, 6*300]

    bcons = np.zeros((128, HT + 1 + 128), np.float32)
    bcons[:, HT + 1:] = np.eye(128, dtype=np.float32)
    w2 = W2.reshape(H)
    hs2 = HSZ[2]
    for t in range(2):
        bcons[:, t] = b1[128 * t:128 * (t + 1)]
    bcons[:hs2, 2] = b1[256:256 + hs2]
    bcons[:, 3] = b2

    # bf16 weight block: diag cols 0:4, then 40 16-col q-blocks (sec0 = t0
    # q 0..15, sec1 = t1, sec2 = t2 pairs m 0..7).
    w2a = np.zeros((128, W2A_W), np.float32)
    w2a[:, 0] = w2[0:128]
    w2a[:, 1] = w2[128:256]
    w2a[:hs2, 2] = w2[256:256 + hs2]
    for q in range(16):
        w2a[:, 4 + (0 + q) * 16 + q] = w2[0:128]
        w2a[:, 4 + (16 + q) * 16 + q] = w2[128:256]
    for m in range(8):
        w2a[:hs2, 4 + (32 + m) * 16 + 2 * m] = w2[256:256 + hs2]
        w2a[64:64 + hs2, 4 + (32 + m) * 16 + 2 * m + 1] = w2[256:256 + hs2]

    b1row = np.zeros((1, 3 * 128), np.float32)
    for t in range(2):
        b1row[0, 128 * t:128 * (t + 1)] = b1[128 * t:128 * (t + 1)]
    b1row[0, 256:256 + hs2] = b1[256:256 + hs2]

    in_maps = []
    for c in range(NCORES):
        sl = slice(c * ISH, (c + 1) * ISH)
        aux = np.concatenate(
            [sbuf_fmt(y[sl].T), sbuf_fmt(x[sl].T), w2a.astype(bf)], axis=1
        )
        in_maps.append({
            "xt_p": xt_p,
            "w1x_p": w1x_p,
            "w1y_p": w1y_p,
            "aux_p": np.ascontiguousarray(aux),
            "bcons": bcons,
            "b1row": np.ascontiguousarray(b1row.astype(bf)),
        })

    nc = _get_module()
    res = run_bass_kernel_spmd(
        nc, in_maps, core_ids=list(range(NCORES)), trace=TRACE
    )
    global LAST_RESULTS
    LAST_RESULTS = res

    # device outS[:, c]: S for local i at row 32c+q of col c, with
    # c = (i//2)%4, q = 2*(i//8) + i%2.
    ii = np.arange(ISH)
    cc = (ii // 2) % 4
    qq = 2 * (ii // 8) + (ii % 2)
    S = np.concatenate(
        [r["outS"].reshape(128, 4)[32 * cc + qq, cc] for r in res.results]
    ).astype(np.float64)
    d = np.concatenate([r["outD"].reshape(ISH) for r in res.results]).astype(np.float64)
    v = d + b2
    t0 = np.log1p(np.exp(-np.abs(v))) + np.maximum(v, 0.0)   # softplus(diag + b2)
    lse = np.log(float(N) + S)
    val = t0.mean() - (lse.mean() - math.log(N))
    return np.float32(val)
